# revision 4
# baseline (speedup 1.0000x reference)
"""Additive (Bahdanau) attention TRN2 kernel — 8 NeuronCores, data-parallel.

Math (per batch b):
    qh = queries[b] @ Wq        (Q, H)
    kh = keys[b] @ Wk           (KV, H)
    scores[q,k] = sum_h wv[h] * tanh(qh[q,h] + kh[k,h])
    out = softmax(mask(scores)) @ values[b]

Device mapping (per core: 2 batches):
    tanh(x) = 1 - 2*sigmoid(-2x), so with scaled projections
    qh2 = -2*q@Wq, kh2 = -2*k@Wk (weights pre-scaled by -2 on host):
      scores = C - 2*sum_h wv[h]*sigmoid(qh2 + kh2),  C = sum(wv)
    - PE: projections (bf16), score reduction as matmuls with a
      sliding-window weight buffer [0..0, -2*wv, 0..0] so each qi's
      (1, KV) score row lands in its own partition of one (64, KV) PSUM
      accumulation (out base-partition must be 32-aligned, M=64 dodges it).
    - DVE: broadcast-add qh2 column onto kh2 tile (tensor_scalar, bf16 4x).
    - ACT: big-tile sigmoids (the only transcendental engine). One table
      set for the whole kernel (softmax also uses sigmoid, see below).
    - softmax without exp: p = e^z = sig(z)/(1-sig(z)); masked scores get
      -1e6 so sig -> 0 -> p = 0. Row sums via fused tensor_tensor_reduce;
      normalization folded into the output copy.
    - valid_lens sparsity: ki chunks of 128 beyond ceil(valid/128) are
      never computed (compile-time per batch-slot, slot-max across cores).
"""

import os
import sys

for _p in ("/opt/trn_rl_repo",):
    if os.path.isdir(_p) and _p not in sys.path:
        sys.path.insert(0, _p)

import numpy as np
import ml_dtypes

from concourse import bacc, bass, mybir, tile
from concourse.bass_utils import run_bass_kernel_spmd

BF = ml_dtypes.bfloat16
DT = mybir.dt
AFT = mybir.ActivationFunctionType
ALU = mybir.AluOpType

B, Q, KV, QS, H, DV = 16, 64, 512, 256, 256, 256
NCORES = 8
SLOTS = B // NCORES  # 2 batches per core

_BUILD_CACHE: dict = {}
LAST_RESULT = None  # BassKernelResults of the most recent run (for test.py)


def _build(nch_slots: tuple) -> "bacc.Bacc":
    nc = bacc.Bacc("TRN2", target_bir_lowering=False, debug=False)

    qT_d = nc.declare_dram_parameter("qT", [SLOTS, QS, Q], DT.bfloat16, isOutput=False)
    kT_d = nc.declare_dram_parameter("kT", [SLOTS, QS, KV], DT.bfloat16, isOutput=False)
    vv_d = nc.declare_dram_parameter("vv", [SLOTS, KV, DV], DT.bfloat16, isOutput=False)
    mk_d = nc.declare_dram_parameter("mk", [SLOTS, Q, KV], DT.float32, isOutput=False)
    wq_d = nc.declare_dram_parameter("wq2", [QS, H], DT.bfloat16, isOutput=False)
    wk_d = nc.declare_dram_parameter("wk2", [QS, H], DT.bfloat16, isOutput=False)
    wvw_d = nc.declare_dram_parameter("wvw", [2, 128, 127], DT.bfloat16, isOutput=False)
    out_d = nc.declare_dram_parameter("out", [SLOTS, Q, DV], DT.float32, isOutput=True)

    with tile.TileContext(nc) as tc:
        with (
            tc.tile_pool(name="const", bufs=1) as constp,
            tc.tile_pool(name="io", bufs=2) as iop,
            tc.tile_pool(name="work", bufs=3) as workp,
            tc.tile_pool(name="sm", bufs=2) as smp,
            tc.tile_pool(name="ps", bufs=2, space="PSUM") as psp,
        ):
            wq_sb = constp.tile([128, 2, H], DT.bfloat16, name="wq_sb")
            nc.sync.dma_start(wq_sb[:], wq_d.ap().rearrange("(c p) h -> p c h", p=128))
            wk_sb = constp.tile([128, 2, H], DT.bfloat16, name="wk_sb")
            nc.sync.dma_start(wk_sb[:], wk_d.ap().rearrange("(c p) h -> p c h", p=128))
            wvw_sb = constp.tile([128, 2, 127], DT.bfloat16, name="wvw_sb")
            nc.sync.dma_start(wvw_sb[:], wvw_d.ap().rearrange("c p w -> p c w"))

            # Pre-warm the sigmoid activation table while input DMAs run.
            warm = constp.tile([1, 8], DT.float32, name="warm")
            warm2 = constp.tile([1, 8], DT.float32, name="warm2")
            nc.vector.memset(warm[:], 0.0)
            nc.scalar.activation(warm2[:], warm[:], AFT.Sigmoid)

            for s in range(SLOTS):
                nch = nch_slots[s]
                W = nch * 128

                qT_sb = iop.tile([128, 2, Q], DT.bfloat16, tag="qT", name="qT_sb")
                nc.sync.dma_start(
                    qT_sb[:], qT_d[s].rearrange("(c p) q -> p c q", p=128)
                )
                kT_sb = iop.tile([128, 2, W], DT.bfloat16, tag="kT", name="kT_sb")
                nc.sync.dma_start(
                    kT_sb[:], kT_d[s].rearrange("(c p) k -> p c k", p=128)[:, :, :W]
                )
                v_sb = iop.tile([128, nch, DV], DT.bfloat16, tag="v", name="v_sb")
                nc.sync.dma_start(
                    v_sb[:], vv_d[s].rearrange("(c p) d -> p c d", p=128)[:, :nch]
                )
                mk_sb = iop.tile([Q, W], DT.float32, tag="mk", name="mk_sb")
                nc.sync.dma_start(mk_sb[:], mk_d[s][:, :W])

                # Projections: qh2 (h, qi) f32, kh2 (h, ki) bf16, h in 2 chunks.
                qh2 = []
                kh2 = []
                for hc in range(2):
                    hsl = slice(hc * 128, (hc + 1) * 128)
                    psq = psp.tile([128, Q], DT.float32, tag="pproj", name="psq")
                    for ksc in range(2):
                        nc.tensor.matmul(
                            psq[:],
                            wq_sb[:, ksc, hsl],
                            qT_sb[:, ksc, :],
                            start=(ksc == 0),
                            stop=(ksc == 1),
                        )
                    qh = workp.tile([128, Q], DT.float32, tag="qh2", bufs=4, name="qh")
                    nc.vector.tensor_copy(qh[:], psq[:])
                    qh2.append(qh)

                    psk = psp.tile([128, W], DT.float32, tag="pproj", name="psk")
                    for ksc in range(2):
                        nc.tensor.matmul(
                            psk[:],
                            wk_sb[:, ksc, hsl],
                            kT_sb[:, ksc, :],
                            start=(ksc == 0),
                            stop=(ksc == 1),
                        )
                    kh = workp.tile([128, W], DT.bfloat16, tag="kh2", bufs=4, name="kh")
                    nc.vector.tensor_copy(kh[:], psk[:])
                    kh2.append(kh)

                # Scores: accumulate all 64 qi rows into one (64, W) PSUM bank.
                ps_s = psp.tile([Q, W], DT.float32, tag="ps_s", name="ps_s")
                first = True
                for g in range(8):  # groups of 8 qi
                    for hc in range(2):
                        s2 = workp.tile([128, 8 * W], DT.bfloat16, tag="s2", name="s2")
                        for j in range(8):
                            qi = g * 8 + j
                            nc.vector.tensor_scalar_add(
                                s2[:, j * W : (j + 1) * W],
                                kh2[hc][:],
                                qh2[hc][:, qi : qi + 1],
                            )
                        rb = workp.tile([128, 8 * W], DT.bfloat16, tag="rb", name="rb")
                        nc.scalar.activation(rb[:], s2[:], AFT.Sigmoid)
                        for j in range(8):
                            qi = g * 8 + j
                            last = g == 7 and hc == 1 and j == 7
                            nc.tensor.matmul(
                                ps_s[:],
                                wvw_sb[:, hc, 63 - qi : 127 - qi],
                                rb[:, j * W : (j + 1) * W],
                                start=first,
                                stop=last,
                            )
                            first = False

                # Masked softmax via sigmoid: p = sig(z) / (1 - sig(z)) = e^z.
                sc = smp.tile([Q, W], DT.float32, tag="sc", name="sc")
                nc.vector.tensor_tensor(sc[:], ps_s[:], mk_sb[:], ALU.add)
                sig = smp.tile([Q, W], DT.float32, tag="sig", name="sig")
                nc.scalar.activation(sig[:], sc[:], AFT.Sigmoid)
                w1 = smp.tile([Q, W], DT.float32, tag="w1", name="w1")
                nc.vector.tensor_scalar(
                    w1[:], sig[:], -1.0, 1.0 + 1e-7, ALU.mult, ALU.add
                )
                rw = smp.tile([Q, W], DT.float32, tag="rw", name="rw")
                nc.vector.reciprocal_approx_fast(rw[:], w1[:])
                p_bf = smp.tile([Q, W], DT.bfloat16, tag="p", name="p_bf")
                nc.vector.tensor_tensor(p_bf[:], sig[:], rw[:], ALU.mult)
                S = smp.tile([Q, 1], DT.float32, tag="S", name="S")
                nc.vector.tensor_reduce(S[:], p_bf[:], mybir.AxisListType.X, ALU.add)
                sinv = smp.tile([Q, 1], DT.float32, tag="sinv", name="sinv")
                nc.vector.reciprocal_approx_fast(sinv[:], S[:])

                # out = (p @ V) * (1/S), p transposed per 128-chunk via DMA xbar.
                ps_o = psp.tile([Q, DV], DT.float32, tag="ps_o", name="ps_o")
                for c in range(nch):
                    pT = workp.tile([128, Q], DT.bfloat16, tag="pT", name="pT")
                    nc.sync.dma_start_transpose(
                        pT[:], p_bf[:, c * 128 : (c + 1) * 128]
                    )
                    nc.tensor.matmul(
                        ps_o[:],
                        pT[:],
                        v_sb[:, c, :],
                        start=(c == 0),
                        stop=(c == nch - 1),
                    )
                ob = smp.tile([Q, DV], DT.float32, tag="ob", name="ob")
                nc.vector.tensor_scalar_mul(ob[:], ps_o[:], sinv[:])
                nc.sync.dma_start(out_d[s], ob[:])

    nc.compile()
    return nc


def kernel(queries, keys, values, valid_lens, Wq, Wk, wv):
    global LAST_RESULT
    queries = np.asarray(queries, dtype=np.float32)
    keys = np.asarray(keys, dtype=np.float32)
    values = np.asarray(values, dtype=np.float32)
    Wq = np.asarray(Wq, dtype=np.float32)
    Wk = np.asarray(Wk, dtype=np.float32)
    wv = np.asarray(wv, dtype=np.float32)
    vl = np.asarray(valid_lens).astype(np.int64)

    # Per-batch live ki chunk counts; sort so slot 0 takes the 8 largest.
    nch = np.maximum(1, -(-vl // 128)).astype(int)  # ceil(vl/128) in 1..4
    order = np.argsort(-nch, kind="stable")
    slots = [order[:NCORES], order[NCORES:][::-1]]
    nch_slots = tuple(int(nch[sl].max()) for sl in slots)

    nc = _BUILD_CACHE.get(nch_slots)
    if nc is None:
        nc = _build(nch_slots)
        _BUILD_CACHE[nch_slots] = nc

    # Host-side input prep: shard, transpose to (feature, token), scale
    # weights by -2, fold C = sum(wv) into the additive mask.
    wq2 = (-2.0 * Wq).astype(BF)
    wk2 = (-2.0 * Wk).astype(BF)
    wvw = np.zeros((2, 128, 127), dtype=BF)
    wvm2 = (-2.0 * wv).astype(BF)
    for hc in range(2):
        wvw[hc, :, 63] = wvm2[hc * 128 : (hc + 1) * 128]
    C = float(wv.sum())

    ki = np.arange(KV)
    in_maps = []
    for core in range(NCORES):
        qT = np.empty((SLOTS, QS, Q), dtype=BF)
        kT = np.empty((SLOTS, QS, KV), dtype=BF)
        vv = np.empty((SLOTS, KV, DV), dtype=BF)
        mk = np.empty((SLOTS, Q, KV), dtype=np.float32)
        for s in range(SLOTS):
            b = int(slots[s][core])
            qT[s] = queries[b].T
            kT[s] = keys[b].T
            vv[s] = values[b]
            mk[s] = np.where(ki < vl[b], C, C - 1e6)[None, :]
        in_maps.append(
            {
                "qT": qT,
                "kT": kT,
                "vv": vv,
                "mk": mk,
                "wq2": wq2,
                "wk2": wk2,
                "wvw": wvw,
            }
        )

    res = run_bass_kernel_spmd(
        nc,
        in_maps,
        core_ids=list(range(NCORES)),
        trace=bool(os.environ.get("KERNEL_TRACE")),
    )
    LAST_RESULT = res

    out = np.empty((B, Q, DV), dtype=np.float32)
    for core in range(NCORES):
        o = res.results[core]["out"]
        for s in range(SLOTS):
            out[int(slots[s][core])] = o[s]
    return out


# revision 8
# speedup vs baseline: 1.6276x; 1.6276x over previous
"""Additive (Bahdanau) attention TRN2 kernel — 8 NeuronCores, data-parallel.

Math (per batch b):
    qh = queries[b] @ Wq   (Q, H);  kh = keys[b] @ Wk   (KV, H)
    scores[q,k] = sum_h wv[h] * tanh(qh[q,h] + kh[k,h])
    out = softmax(mask(scores)) @ values[b]

Key trick: tanh(x) ~= sum_j c_j sin(om_j x) on [-7, 7] (R=8 harmonics,
sup err ~1e-2, rel contribution ~4e-3), and sin separates over x = a + b:
    sin(om(a+b)) = sin(om a)cos(om b) + cos(om a)sin(om b)
so the (Q, KV, H) tanh tensor is never materialized. Per frequency the
kernel computes sin/cos of the SMALL projected tensors (ACT engine, with
DVE range-reduction into [-pi, pi] via one fused mul pass + at most one
add_range_wrap), then reduces over h with 2R accumulated PE matmuls into
a (64, KV) PSUM scores tile (wv and c_j folded into the q-side weights).
cos is obtained as -cos(w) = sin(|w| - pi/2) so one big ACT Sin per
frequency covers both halves. Projections are clamped to +-3.5 (clamp
error ~1e-4) so one wrap suffices at om_max = 2.69.

Softmax without max-subtraction (scores are O(1)): p = Exp(scores+mask)
with the row sum taken by the same ACT instruction (accum_out); masked
columns give exp(-1e6) = 0. Row normalization is folded into the output
copy. valid_lens sparsity: ki chunks of 128 beyond ceil(valid/128) are
skipped at compile time (per batch-slot, slot-max across cores; batches
sorted so heavy ones share a slot).
"""

import os
import sys

for _p in ("/opt/trn_rl_repo",):
    if os.path.isdir(_p) and _p not in sys.path:
        sys.path.insert(0, _p)

import numpy as np
import ml_dtypes

from concourse import bacc, bass, mybir, tile
from concourse.bass_utils import run_bass_kernel_spmd

BF = ml_dtypes.bfloat16
DT = mybir.dt
AFT = mybir.ActivationFunctionType
ALU = mybir.AluOpType

B, Q, KV, QS, H, DV = 16, 64, 512, 256, 256, 256
NCORES = 8
SLOTS = B // NCORES  # 2 batches per core

CLAMP = 3.5
NFREQ = 8
_L = 9.33
OMEGA = (np.arange(1, NFREQ + 1) * np.pi / _L).astype(np.float64)
_xg = np.linspace(-2 * CLAMP, 2 * CLAMP, 8001)
_A = np.sin(np.outer(_xg, OMEGA))
COEF, *_ = np.linalg.lstsq(_A, np.tanh(_xg), rcond=None)
PI = float(np.pi)
TWO_PI = float(2 * np.pi)
HALF_PI = float(np.pi / 2)

_BUILD_CACHE: dict = {}
LAST_RESULT = None  # BassKernelResults of the most recent run (for test.py)


def _build(nch_slots: tuple) -> "bacc.Bacc":
    nc = bacc.Bacc("TRN2", target_bir_lowering=False, debug=False)

    qT_d = nc.declare_dram_parameter("qT", [SLOTS, QS, Q], DT.bfloat16, isOutput=False)
    kT_d = nc.declare_dram_parameter("kT", [SLOTS, QS, KV], DT.bfloat16, isOutput=False)
    vv_d = nc.declare_dram_parameter("vv", [SLOTS, KV, DV], DT.bfloat16, isOutput=False)
    mk_d = nc.declare_dram_parameter("mk", [SLOTS, Q, KV], DT.float32, isOutput=False)
    wq_d = nc.declare_dram_parameter("wq", [QS, H], DT.bfloat16, isOutput=False)
    wk_d = nc.declare_dram_parameter("wk", [QS, H], DT.bfloat16, isOutput=False)
    wvc_d = nc.declare_dram_parameter("wvc", [128, 2, NFREQ], DT.float32, isOutput=False)
    out_d = nc.declare_dram_parameter("out", [SLOTS, Q, DV], DT.float32, isOutput=True)

    with tile.TileContext(nc) as tc:
        with (
            tc.tile_pool(name="const", bufs=1) as constp,
            tc.tile_pool(name="io", bufs=2) as iop,
            tc.tile_pool(name="work", bufs=3) as workp,
            tc.tile_pool(name="sm", bufs=2) as smp,
            tc.tile_pool(name="ps", bufs=2, space="PSUM") as psp,
        ):
            wq_sb = constp.tile([128, 2, H], DT.bfloat16, name="wq_sb")
            nc.sync.dma_start(wq_sb[:], wq_d.ap().rearrange("(c p) h -> p c h", p=128))
            wk_sb = constp.tile([128, 2, H], DT.bfloat16, name="wk_sb")
            nc.sync.dma_start(wk_sb[:], wk_d.ap().rearrange("(c p) h -> p c h", p=128))
            wvc_sb = constp.tile([128, 2, NFREQ], DT.float32, name="wvc_sb")
            nc.sync.dma_start(wvc_sb[:], wvc_d.ap())

            # Pre-warm the sin activation table while input DMAs run.
            _salt = os.environ.get("KERNEL_SALT", "")
            warm = constp.tile([1, 8], DT.float32, name=f"warm{_salt}")
            warm2 = constp.tile([1, 8], DT.float32, name="warm2")
            nc.vector.memset(warm[:], 0.0)
            nc.scalar.activation(warm2[:], warm[:], AFT.Sin)
            nhpi = constp.tile([128, 1], DT.float32, name="nhpi")
            nc.vector.memset(nhpi[:], -HALF_PI)

            slot_state = []
            # ---- Phase A per slot: projections, sin/cos factors, scores ----
            for s in range(SLOTS):
                nch = nch_slots[s]
                W = nch * 128

                qT_sb = iop.tile([128, 2, Q], DT.bfloat16, tag="qT", name="qT_sb")
                nc.sync.dma_start(
                    qT_sb[:], qT_d[s].rearrange("(c p) q -> p c q", p=128)
                )
                kT_sb = iop.tile([128, 2, W], DT.bfloat16, tag="kT", name="kT_sb")
                nc.sync.dma_start(
                    kT_sb[:], kT_d[s].rearrange("(c p) k -> p c k", p=128)[:, :, :W]
                )
                v_sb = iop.tile([128, nch, DV], DT.bfloat16, tag="v", name="v_sb")
                nc.sync.dma_start(
                    v_sb[:], vv_d[s].rearrange("(c p) d -> p c d", p=128)[:, :nch]
                )
                mk_sb = iop.tile([Q, W], DT.float32, tag="mk", name="mk_sb")
                nc.sync.dma_start(mk_sb[:], mk_d[s][:, :W])

                # Projections (PSUM f32) then clamp to +-CLAMP into SBUF f32.
                qc = iop.tile([128, 2, Q], DT.float32, tag="qc", name="qc")
                kc = iop.tile([128, 2, W], DT.float32, tag="kc", name="kc")
                for hc in range(2):
                    hsl = slice(hc * 128, (hc + 1) * 128)
                    psq = psp.tile([128, Q], DT.float32, tag="pproj", name="psq")
                    for ksc in range(2):
                        nc.tensor.matmul(
                            psq[:],
                            wq_sb[:, ksc, hsl],
                            qT_sb[:, ksc, :],
                            start=(ksc == 0),
                            stop=(ksc == 1),
                        )
                    nc.vector.tensor_scalar(
                        qc[:, hc, :], psq[:], -CLAMP, CLAMP, ALU.max, ALU.min
                    )
                    psk = psp.tile([128, W], DT.float32, tag="pproj", name="psk")
                    for ksc in range(2):
                        nc.tensor.matmul(
                            psk[:],
                            wk_sb[:, ksc, hsl],
                            kT_sb[:, ksc, :],
                            start=(ksc == 0),
                            stop=(ksc == 1),
                        )
                    nc.vector.tensor_scalar(
                        kc[:, hc, :], psk[:], -CLAMP, CLAMP, ALU.max, ALU.min
                    )

                # Scores accumulate into one (64, W) PSUM tile across 2R*2 mms.
                ps_s = psp.tile([Q, W], DT.float32, tag="ps_s", name="ps_s")
                first = True
                for j in range(NFREQ):
                    om = float(OMEGA[j])
                    need_wrap = om * CLAMP > PI

                    # A side: args (128, [fn=2][hc=2][Q]) f32
                    aarg = workp.tile([128, 2, 2, Q], DT.float32, tag="aarg",
                                      name="aarg")
                    nc.vector.tensor_scalar_mul(
                        aarg[:, 0, :, :].rearrange("p c q -> p (c q)"),
                        qc[:].rearrange("p c q -> p (c q)"), om)
                    if need_wrap:
                        nc.vector.add_range_wrap(
                            aarg[:, 0, :, :].rearrange("p c q -> p (c q)"),
                            aarg[:, 0, :, :].rearrange("p c q -> p (c q)"),
                            shift=0.0, bound=PI, period=TWO_PI)
                    nc.vector.tensor_scalar(
                        aarg[:, 1, :, :].rearrange("p c q -> p (c q)").bitcast(DT.int32),
                        aarg[:, 0, :, :].rearrange("p c q -> p (c q)").bitcast(DT.int32),
                        0x7FFFFFFF, None, ALU.bitwise_and)
                    aval = workp.tile([128, 2, 2, Q], DT.float32, tag="aval",
                                      name="aval")
                    nc.scalar.activation(
                        aval[:, 0, :, :].rearrange("p c q -> p (c q)"),
                        aarg[:, 0, :, :].rearrange("p c q -> p (c q)"), AFT.Sin)
                    nc.scalar.activation(
                        aval[:, 1, :, :].rearrange("p c q -> p (c q)"),
                        aarg[:, 1, :, :].rearrange("p c q -> p (c q)"), AFT.Sin,
                        bias=nhpi[:])
                    # fold -c_j * wv into the q side -> bf16 weights
                    att = workp.tile([128, 2, 2, Q], DT.bfloat16, tag="att",
                                     name="att")
                    for fn in range(2):
                        for hc in range(2):
                            nc.vector.tensor_scalar_mul(
                                att[:, fn, hc, :], aval[:, fn, hc, :],
                                wvc_sb[:, hc, j : j + 1])

                    # B side: args (128, [fn=2][hc=2][W]) f32
                    barg = workp.tile([128, 2, 2, W], DT.float32, tag="barg",
                                      name="barg")
                    nc.vector.tensor_scalar_mul(
                        barg[:, 0, :, :].rearrange("p c k -> p (c k)"),
                        kc[:].rearrange("p c k -> p (c k)"), om)
                    if need_wrap:
                        nc.vector.add_range_wrap(
                            barg[:, 0, :, :].rearrange("p c k -> p (c k)"),
                            barg[:, 0, :, :].rearrange("p c k -> p (c k)"),
                            shift=0.0, bound=PI, period=TWO_PI)
                    nc.vector.tensor_scalar(
                        barg[:, 1, :, :].rearrange("p c k -> p (c k)").bitcast(DT.int32),
                        barg[:, 0, :, :].rearrange("p c k -> p (c k)").bitcast(DT.int32),
                        0x7FFFFFFF, None, ALU.bitwise_and)
                    bval = workp.tile([128, 2, 2, W], DT.bfloat16, tag="bval",
                                      name="bval")
                    nc.scalar.activation(
                        bval[:, 0, :, :].rearrange("p c k -> p (c k)"),
                        barg[:, 0, :, :].rearrange("p c k -> p (c k)"), AFT.Sin)
                    nc.scalar.activation(
                        bval[:, 1, :, :].rearrange("p c k -> p (c k)"),
                        barg[:, 1, :, :].rearrange("p c k -> p (c k)"), AFT.Sin,
                        bias=nhpi[:])

                    # sin(om(a+b)) = -[sin_a*(-cos_b) + (-cos_a)*sin_b]; the
                    # minus is folded into wvc = -c_j*wv.
                    for hc in range(2):
                        nc.tensor.matmul(
                            ps_s[:], att[:, 0, hc, :], bval[:, 1, hc, :],
                            start=first, stop=False)
                        first = False
                        last = j == NFREQ - 1 and hc == 1
                        nc.tensor.matmul(
                            ps_s[:], att[:, 1, hc, :], bval[:, 0, hc, :],
                            start=False, stop=last)
                slot_state.append((nch, W, ps_s, mk_sb, v_sb))

            # ---- Phase B per slot: softmax (Exp) + attn @ V + store ----
            for s in range(SLOTS):
                nch, W, ps_s, mk_sb, v_sb = slot_state[s]
                sc = smp.tile([Q, W], DT.float32, tag="sc", name="sc")
                nc.vector.tensor_tensor(sc[:], ps_s[:], mk_sb[:], ALU.add)
                p_bf = smp.tile([Q, W], DT.bfloat16, tag="p", name="p_bf")
                S = smp.tile([Q, 1], DT.float32, tag="S", name="S")
                nc.scalar.activation(p_bf[:], sc[:], AFT.Exp, accum_out=S[:])
                sinv = smp.tile([Q, 1], DT.float32, tag="sinv", name="sinv")
                nc.vector.reciprocal_approx_fast(sinv[:], S[:])

                ps_o = psp.tile([Q, DV], DT.float32, tag="ps_o", name="ps_o")
                for c in range(nch):
                    pT = workp.tile([128, Q], DT.bfloat16, tag="pT", name="pT")
                    nc.sync.dma_start_transpose(
                        pT[:], p_bf[:, c * 128 : (c + 1) * 128]
                    )
                    nc.tensor.matmul(
                        ps_o[:], pT[:], v_sb[:, c, :],
                        start=(c == 0), stop=(c == nch - 1),
                    )
                ob = smp.tile([Q, DV], DT.float32, tag="ob", name="ob")
                nc.vector.tensor_scalar_mul(ob[:], ps_o[:], sinv[:])
                nc.sync.dma_start(out_d[s], ob[:])

    nc.compile()
    return nc


def kernel(queries, keys, values, valid_lens, Wq, Wk, wv):
    global LAST_RESULT
    queries = np.asarray(queries, dtype=np.float32)
    keys = np.asarray(keys, dtype=np.float32)
    values = np.asarray(values, dtype=np.float32)
    Wq = np.asarray(Wq, dtype=np.float32)
    Wk = np.asarray(Wk, dtype=np.float32)
    wv = np.asarray(wv, dtype=np.float32)
    vl = np.asarray(valid_lens).astype(np.int64)

    # Per-batch live ki chunk counts; sort so slot 0 takes the 8 largest.
    nch = np.maximum(1, -(-vl // 128)).astype(int)  # ceil(vl/128) in 1..4
    order = np.argsort(-nch, kind="stable")
    slots = [order[:NCORES], order[NCORES:][::-1]]
    nch_slots = tuple(int(nch[sl].max()) for sl in slots)

    nc = _BUILD_CACHE.get(nch_slots)
    if nc is None:
        nc = _build(nch_slots)
        _BUILD_CACHE[nch_slots] = nc

    wq16 = Wq.astype(BF)
    wk16 = Wk.astype(BF)
    wvc = np.empty((128, 2, NFREQ), np.float32)
    for hc in range(2):
        for j in range(NFREQ):
            wvc[:, hc, j] = -float(COEF[j]) * wv[hc * 128 : (hc + 1) * 128]

    ki = np.arange(KV)
    in_maps = []
    for core in range(NCORES):
        qT = np.empty((SLOTS, QS, Q), dtype=BF)
        kT = np.empty((SLOTS, QS, KV), dtype=BF)
        vvv = np.empty((SLOTS, KV, DV), dtype=BF)
        mk = np.empty((SLOTS, Q, KV), dtype=np.float32)
        for s in range(SLOTS):
            b = int(slots[s][core])
            qT[s] = queries[b].T
            kT[s] = keys[b].T
            vvv[s] = values[b]
            mk[s] = np.where(ki < vl[b], 0.0, -1e6)[None, :]
        in_maps.append(
            {"qT": qT, "kT": kT, "vv": vvv, "mk": mk,
             "wq": wq16, "wk": wk16, "wvc": wvc}
        )

    res = run_bass_kernel_spmd(
        nc,
        in_maps,
        core_ids=list(range(NCORES)),
        trace=bool(os.environ.get("KERNEL_TRACE")),
    )
    LAST_RESULT = res

    out = np.empty((B, Q, DV), dtype=np.float32)
    for core in range(NCORES):
        o = res.results[core]["out"]
        for s in range(SLOTS):
            out[int(slots[s][core])] = o[s]
    return out


# revision 10
# speedup vs baseline: 1.6370x; 1.0057x over previous
"""Additive (Bahdanau) attention TRN2 kernel — 8 NeuronCores, data-parallel.

Math (per batch b):
    qh = queries[b] @ Wq   (Q, H);  kh = keys[b] @ Wk   (KV, H)
    scores[q,k] = sum_h wv[h] * tanh(qh[q,h] + kh[k,h])
    out = softmax(mask(scores)) @ values[b]

Key trick: tanh(x) ~= sum_j c_j sin(om_j x) on [-7, 7] (R=8 harmonics,
sup err ~1e-2, rel contribution ~4e-3), and sin separates over x = a + b:
    sin(om(a+b)) = sin(om a)cos(om b) + cos(om a)sin(om b)
so the (Q, KV, H) tanh tensor is never materialized. Per frequency the
kernel computes sin/cos of the SMALL projected tensors (ACT engine, with
DVE range-reduction into [-pi, pi] via one fused mul pass + at most one
add_range_wrap), then reduces over h with 2R accumulated PE matmuls into
a (64, KV) PSUM scores tile (wv and c_j folded into the q-side weights).
cos is obtained as -cos(w) = sin(|w| - pi/2) so one big ACT Sin per
frequency covers both halves. Projections are clamped to +-3.5 (clamp
error ~1e-4) so one wrap suffices at om_max = 2.69.

Softmax without max-subtraction (scores are O(1)): p = Exp(scores+mask)
with the row sum taken by the same ACT instruction (accum_out); masked
columns give exp(-1e6) = 0. Row normalization is folded into the output
copy. valid_lens sparsity: ki chunks of 128 beyond ceil(valid/128) are
skipped at compile time (per batch-slot, slot-max across cores; batches
sorted so heavy ones share a slot).
"""

import os
import sys

for _p in ("/opt/trn_rl_repo",):
    if os.path.isdir(_p) and _p not in sys.path:
        sys.path.insert(0, _p)

import numpy as np
import ml_dtypes

from concourse import bacc, bass, mybir, tile
from concourse.bass_utils import run_bass_kernel_spmd

BF = ml_dtypes.bfloat16
DT = mybir.dt
AFT = mybir.ActivationFunctionType
ALU = mybir.AluOpType

B, Q, KV, QS, H, DV = 16, 64, 512, 256, 256, 256
NCORES = 8
SLOTS = B // NCORES  # 2 batches per core

CLAMP = 3.5
NFREQ = 8
_L = 9.33
OMEGA = (np.arange(1, NFREQ + 1) * np.pi / _L).astype(np.float64)
_xg = np.linspace(-2 * CLAMP, 2 * CLAMP, 8001)
_A = np.sin(np.outer(_xg, OMEGA))
COEF, *_ = np.linalg.lstsq(_A, np.tanh(_xg), rcond=None)
PI = float(np.pi)
TWO_PI = float(2 * np.pi)
HALF_PI = float(np.pi / 2)

_BUILD_CACHE: dict = {}
LAST_RESULT = None  # BassKernelResults of the most recent run (for test.py)


def _build(nch_slots: tuple) -> "bacc.Bacc":
    nc = bacc.Bacc("TRN2", target_bir_lowering=False, debug=False)

    qT_d = nc.declare_dram_parameter("qT", [SLOTS, QS, Q], DT.bfloat16, isOutput=False)
    kT_d = nc.declare_dram_parameter("kT", [SLOTS, QS, KV], DT.bfloat16, isOutput=False)
    vv_d = nc.declare_dram_parameter("vv", [SLOTS, KV, DV], DT.bfloat16, isOutput=False)
    mk_d = nc.declare_dram_parameter("mk", [SLOTS, Q, KV], DT.float32, isOutput=False)
    wq_d = nc.declare_dram_parameter("wq", [QS, H], DT.bfloat16, isOutput=False)
    wk_d = nc.declare_dram_parameter("wk", [QS, H], DT.bfloat16, isOutput=False)
    wvc_d = nc.declare_dram_parameter("wvc", [128, 2, NFREQ], DT.float32, isOutput=False)
    out_d = nc.declare_dram_parameter("out", [SLOTS, Q, DV], DT.float32, isOutput=True)

    with tile.TileContext(nc) as tc:
        with (
            tc.tile_pool(name="const", bufs=1) as constp,
            tc.tile_pool(name="io", bufs=2) as iop,
            tc.tile_pool(name="work", bufs=3) as workp,
            tc.tile_pool(name="sm", bufs=2) as smp,
            tc.tile_pool(name="ps", bufs=2, space="PSUM") as psp,
        ):
            wq_sb = constp.tile([128, 2, H], DT.bfloat16, name="wq_sb")
            nc.sync.dma_start(wq_sb[:], wq_d.ap().rearrange("(c p) h -> p c h", p=128))
            wk_sb = constp.tile([128, 2, H], DT.bfloat16, name="wk_sb")
            nc.sync.dma_start(wk_sb[:], wk_d.ap().rearrange("(c p) h -> p c h", p=128))
            wvc_sb = constp.tile([128, 2, NFREQ], DT.float32, name="wvc_sb")
            nc.sync.dma_start(wvc_sb[:], wvc_d.ap())

            # Pre-warm the sin activation table while input DMAs run.
            _salt = os.environ.get("KERNEL_SALT", "")
            warm = constp.tile([1, 8], DT.float32, name=f"warm{_salt}")
            warm2 = constp.tile([1, 8], DT.float32, name="warm2")
            nc.vector.memset(warm[:], 0.0)
            nc.scalar.activation(warm2[:], warm[:], AFT.Sin)
            nhpi = constp.tile([128, 1], DT.float32, name="nhpi")
            nc.vector.memset(nhpi[:], -HALF_PI)

            slot_state = []
            # ---- Phase A per slot: projections, sin/cos factors, scores ----
            for s in range(SLOTS):
                nch = nch_slots[s]
                W = nch * 128

                qT_sb = iop.tile([128, 2, Q], DT.bfloat16, tag="qT", name="qT_sb")
                nc.sync.dma_start(
                    qT_sb[:], qT_d[s].rearrange("(c p) q -> p c q", p=128)
                )
                kT_sb = iop.tile([128, 2, W], DT.bfloat16, tag="kT", name="kT_sb")
                nc.sync.dma_start(
                    kT_sb[:], kT_d[s].rearrange("(c p) k -> p c k", p=128)[:, :, :W]
                )
                v_sb = iop.tile([128, nch, DV], DT.bfloat16, tag="v", name="v_sb")
                nc.sync.dma_start(
                    v_sb[:], vv_d[s].rearrange("(c p) d -> p c d", p=128)[:, :nch]
                )
                mk_sb = iop.tile([Q, W], DT.float32, tag="mk", name="mk_sb")
                nc.sync.dma_start(mk_sb[:], mk_d[s][:, :W])

                # Projections (PSUM f32) then clamp to +-CLAMP into SBUF f32.
                qc = iop.tile([128, 2, Q], DT.float32, tag="qc", name="qc")
                kc = iop.tile([128, 2, W], DT.float32, tag="kc", name="kc")
                for hc in range(2):
                    hsl = slice(hc * 128, (hc + 1) * 128)
                    psq = psp.tile([128, Q], DT.float32, tag="pproj", name="psq")
                    for ksc in range(2):
                        nc.tensor.matmul(
                            psq[:],
                            wq_sb[:, ksc, hsl],
                            qT_sb[:, ksc, :],
                            start=(ksc == 0),
                            stop=(ksc == 1),
                        )
                    nc.vector.tensor_scalar(
                        qc[:, hc, :], psq[:], -CLAMP, CLAMP, ALU.max, ALU.min
                    )
                    psk = psp.tile([128, W], DT.float32, tag="pproj", name="psk")
                    for ksc in range(2):
                        nc.tensor.matmul(
                            psk[:],
                            wk_sb[:, ksc, hsl],
                            kT_sb[:, ksc, :],
                            start=(ksc == 0),
                            stop=(ksc == 1),
                        )
                    nc.vector.tensor_scalar(
                        kc[:, hc, :], psk[:], -CLAMP, CLAMP, ALU.max, ALU.min
                    )

                # A side hoisted: args/sins/weights for ALL frequencies up
                # front so the scores-matmul LDWEIGHTS never waits on DVE.
                aarg = iop.tile([128, NFREQ, 2, 2 * Q], DT.float32, tag="aarg",
                                name="aarg")
                for j in range(NFREQ):
                    om = float(OMEGA[j])
                    nc.vector.tensor_scalar_mul(
                        aarg[:, j, 0, :], qc[:].rearrange("p c q -> p (c q)"), om)
                    if om * CLAMP > PI:
                        nc.vector.add_range_wrap(
                            aarg[:, j, 0, :], aarg[:, j, 0, :],
                            shift=0.0, bound=PI, period=TWO_PI)
                    nc.vector.tensor_scalar(
                        aarg[:, j, 1, :].bitcast(DT.int32),
                        aarg[:, j, 0, :].bitcast(DT.int32),
                        0x7FFFFFFF, None, ALU.bitwise_and)
                aval = iop.tile([128, NFREQ, 2, 2 * Q], DT.float32, tag="aval",
                                name="aval")
                nc.scalar.activation(aval[:, :, 0, :], aarg[:, :, 0, :], AFT.Sin)
                nc.scalar.activation(aval[:, :, 1, :], aarg[:, :, 1, :], AFT.Sin,
                                     bias=nhpi[:])
                att = iop.tile([128, NFREQ, 2, 2, Q], DT.bfloat16, tag="att",
                               name="att")
                for j in range(NFREQ):
                    for hc in range(2):
                        nc.vector.tensor_scalar_mul(
                            att[:, j, :, hc, :],
                            aval[:, j, :, hc * Q : (hc + 1) * Q].rearrange(
                                "p f q -> p f q"),
                            wvc_sb[:, hc, j : j + 1])

                # Scores accumulate into one (64, W) PSUM tile across 2R*2 mms.
                ps_s = psp.tile([Q, W], DT.float32, tag="ps_s", name="ps_s")
                first = True
                for j in range(NFREQ):
                    om = float(OMEGA[j])
                    need_wrap = om * CLAMP > PI

                    # B side: args (128, [fn=2][hc=2][W]) f32
                    barg = workp.tile([128, 2, 2, W], DT.float32, tag="barg",
                                      name="barg")
                    nc.vector.tensor_scalar_mul(
                        barg[:, 0, :, :].rearrange("p c k -> p (c k)"),
                        kc[:].rearrange("p c k -> p (c k)"), om)
                    if need_wrap:
                        nc.vector.add_range_wrap(
                            barg[:, 0, :, :].rearrange("p c k -> p (c k)"),
                            barg[:, 0, :, :].rearrange("p c k -> p (c k)"),
                            shift=0.0, bound=PI, period=TWO_PI)
                    _abs_eng = nc.gpsimd if os.environ.get(
                        "KERNEL_GP_ABS", "0") != "0" else nc.vector
                    _abs_eng.tensor_scalar(
                        barg[:, 1, :, :].rearrange("p c k -> p (c k)").bitcast(DT.int32),
                        barg[:, 0, :, :].rearrange("p c k -> p (c k)").bitcast(DT.int32),
                        0x7FFFFFFF, None, ALU.bitwise_and)
                    bval = workp.tile([128, 2, 2, W], DT.bfloat16, tag="bval",
                                      name="bval")
                    nc.scalar.activation(
                        bval[:, 0, :, :].rearrange("p c k -> p (c k)"),
                        barg[:, 0, :, :].rearrange("p c k -> p (c k)"), AFT.Sin)
                    nc.scalar.activation(
                        bval[:, 1, :, :].rearrange("p c k -> p (c k)"),
                        barg[:, 1, :, :].rearrange("p c k -> p (c k)"), AFT.Sin,
                        bias=nhpi[:])

                    # sin(om(a+b)) = -[sin_a*(-cos_b) + (-cos_a)*sin_b]; the
                    # minus is folded into wvc = -c_j*wv.
                    for hc in range(2):
                        nc.tensor.matmul(
                            ps_s[:], att[:, j, 0, hc, :], bval[:, 1, hc, :],
                            start=first, stop=False)
                        first = False
                        last = j == NFREQ - 1 and hc == 1
                        nc.tensor.matmul(
                            ps_s[:], att[:, j, 1, hc, :], bval[:, 0, hc, :],
                            start=False, stop=last)
                slot_state.append((nch, W, ps_s, mk_sb, v_sb))

            # ---- Phase B per slot: softmax (Exp) + attn @ V + store ----
            for s in range(SLOTS):
                nch, W, ps_s, mk_sb, v_sb = slot_state[s]
                sc = smp.tile([Q, W], DT.float32, tag="sc", name="sc")
                nc.vector.tensor_tensor(sc[:], ps_s[:], mk_sb[:], ALU.add)
                p_bf = smp.tile([Q, W], DT.bfloat16, tag="p", name="p_bf")
                S = smp.tile([Q, 1], DT.float32, tag="S", name="S")
                nc.scalar.activation(p_bf[:], sc[:], AFT.Exp, accum_out=S[:])
                sinv = smp.tile([Q, 1], DT.float32, tag="sinv", name="sinv")
                nc.vector.reciprocal_approx_fast(sinv[:], S[:])

                ps_o = psp.tile([Q, DV], DT.float32, tag="ps_o", name="ps_o")
                for c in range(nch):
                    pT = workp.tile([128, Q], DT.bfloat16, tag="pT", name="pT")
                    nc.sync.dma_start_transpose(
                        pT[:], p_bf[:, c * 128 : (c + 1) * 128]
                    )
                    nc.tensor.matmul(
                        ps_o[:], pT[:], v_sb[:, c, :],
                        start=(c == 0), stop=(c == nch - 1),
                    )
                ob = smp.tile([Q, DV], DT.float32, tag="ob", name="ob")
                nc.vector.tensor_scalar_mul(ob[:], ps_o[:], sinv[:])
                nc.sync.dma_start(out_d[s], ob[:])

    nc.compile()
    return nc


def kernel(queries, keys, values, valid_lens, Wq, Wk, wv):
    global LAST_RESULT
    queries = np.asarray(queries, dtype=np.float32)
    keys = np.asarray(keys, dtype=np.float32)
    values = np.asarray(values, dtype=np.float32)
    Wq = np.asarray(Wq, dtype=np.float32)
    Wk = np.asarray(Wk, dtype=np.float32)
    wv = np.asarray(wv, dtype=np.float32)
    vl = np.asarray(valid_lens).astype(np.int64)

    # Per-batch live ki chunk counts; sort so slot 0 takes the 8 largest.
    nch = np.maximum(1, -(-vl // 128)).astype(int)  # ceil(vl/128) in 1..4
    order = np.argsort(-nch, kind="stable")
    slots = [order[:NCORES], order[NCORES:][::-1]]
    nch_slots = tuple(int(nch[sl].max()) for sl in slots)

    nc = _BUILD_CACHE.get(nch_slots)
    if nc is None:
        nc = _build(nch_slots)
        _BUILD_CACHE[nch_slots] = nc

    wq16 = Wq.astype(BF)
    wk16 = Wk.astype(BF)
    wvc = np.empty((128, 2, NFREQ), np.float32)
    for hc in range(2):
        for j in range(NFREQ):
            wvc[:, hc, j] = -float(COEF[j]) * wv[hc * 128 : (hc + 1) * 128]

    ki = np.arange(KV)
    in_maps = []
    for core in range(NCORES):
        qT = np.empty((SLOTS, QS, Q), dtype=BF)
        kT = np.empty((SLOTS, QS, KV), dtype=BF)
        vvv = np.empty((SLOTS, KV, DV), dtype=BF)
        mk = np.empty((SLOTS, Q, KV), dtype=np.float32)
        for s in range(SLOTS):
            b = int(slots[s][core])
            qT[s] = queries[b].T
            kT[s] = keys[b].T
            vvv[s] = values[b]
            mk[s] = np.where(ki < vl[b], 0.0, -1e6)[None, :]
        in_maps.append(
            {"qT": qT, "kT": kT, "vv": vvv, "mk": mk,
             "wq": wq16, "wk": wk16, "wvc": wvc}
        )

    res = run_bass_kernel_spmd(
        nc,
        in_maps,
        core_ids=list(range(NCORES)),
        trace=bool(os.environ.get("KERNEL_TRACE")),
    )
    LAST_RESULT = res

    out = np.empty((B, Q, DV), dtype=np.float32)
    for core in range(NCORES):
        o = res.results[core]["out"]
        for s in range(SLOTS):
            out[int(slots[s][core])] = o[s]
    return out
